# revision 46
# baseline (speedup 1.0000x reference)
"""Distributed causal self-attention for 8 Trainium2 NeuronCores.

Problem: x[2,2048,1024] @ w_qkv[1024,3072] -> causal MHA (16 heads, d=64)
         -> @ w_out[1024,1024]. All fp32.

Sharding: core c (0..7) handles batch b=c//4 and head group g=c%4 (4 heads).
Each core projects qkv for its heads, runs flash attention (transposed-score
layout), then a per-chunk ReduceScatter within each 4-core batch group sums
the partial output projections. Core c owns output rows [b, qc*512+g*128 ..
+128) for each query chunk qc.

Pipeline layout (v2):
- x/weights DMA'd as bf16, k-chunk-split so projection matmuls chase the
  loads (no serial load phase).
- qkv projection kk-outer with 8 PSUM groups in flight.
- attention per (qc, head-pair): per-key-block units; the two heads' score
  matmuls occupy disjoint PE row groups (tile_position packing) and share
  one merged exp; softmax denominator rides as a ones-column in v.
- output projection partials (bias/4 pre-added, bf16) stream to DRAM; the
  ReduceScatter is issued as a fire-and-forget job and its result is copied
  DRAM->DRAM to the bf16 external output one chunk later, so nothing ever
  head-blocks a compute queue on collective latency.
"""

import sys

for _p in ("/opt/trn_rl_repo", "/root/.axon_site/_ro/trn_rl_repo"):
    if _p not in sys.path:
        sys.path.insert(0, _p)

import numpy as np
import ml_dtypes

import concourse.bass as bass  # noqa: F401  (bass types used via tile/bacc)
import concourse.mybir as mybir
import concourse.tile as tile
from concourse import bacc
from concourse.bass_utils import run_bass_kernel_spmd

P = 128
B, T, C = 2, 2048, 1024
H, D = 16, 64
HL = 4               # heads per core
DL = HL * D          # 256 local head dims
KC = C // P          # 8 contraction tiles over C
QB = 512             # query chunk
NQ = T // QB         # 4 query chunks
NT = T // P          # 16 token tiles
G = 4                # cores per batch group
SCALE = 1.0 / 8.0    # 1/sqrt(64)
NEG = -1.0e30

F32 = mybir.dt.float32
F32R = mybir.dt.float32r
BF16 = mybir.dt.bfloat16

_CACHED = {}
LAST_RESULT = None  # BassKernelResults of the most recent kernel() call


def _mask_data():
    # tril mask: 0 where key j <= query i, NEG above the diagonal
    j = np.arange(P)[:, None]
    i = np.arange(P)[None, :]
    return np.where(j <= i, 0.0, NEG).astype(np.float32)


def _build(debug_taps=False):
    nc = bacc.Bacc("TRN2", target_bir_lowering=False, debug=False,
                   num_devices=8)

    xT = nc.dram_tensor("xT", [C, T], BF16, kind="ExternalInput")
    # weights arrive host-pre-transposed to the SBUF layout so the DMA is
    # fully contiguous (4KB runs per partition vs 512B strided gathers)
    wq = nc.dram_tensor("wq", [P, KC * DL], BF16, kind="ExternalInput")
    wk = nc.dram_tensor("wk", [P, KC * DL], BF16, kind="ExternalInput")
    wv = nc.dram_tensor("wv", [P, KC * DL], BF16, kind="ExternalInput")
    bq = nc.dram_tensor("bq", [1, DL], F32, kind="ExternalInput")
    bk = nc.dram_tensor("bk", [1, DL], F32, kind="ExternalInput")
    bv = nc.dram_tensor("bv", [1, DL], F32, kind="ExternalInput")
    wo = nc.dram_tensor("wo", [P, 2 * C], BF16, kind="ExternalInput")
    bo4 = nc.dram_tensor("bo4", [1, C], F32, kind="ExternalInput")
    # per query-chunk ReduceScatter slices: rows qc*512 + g*128 .. +128
    out = nc.dram_tensor("out", [NQ, P, C], BF16, kind="ExternalOutput")
    if debug_taps:
        dbg_q = nc.dram_tensor("dbg_q", [P, 2, T], F32, kind="ExternalOutput")
        dbg_k = nc.dram_tensor("dbg_k", [P, 2, T], F32, kind="ExternalOutput")
        dbg_v = nc.dram_tensor("dbg_v", [P, NT, HL * (D + 1)], F32,
                               kind="ExternalOutput")
        dbg_ao = nc.dram_tensor("dbg_ao", [P, 2, T], F32,
                                kind="ExternalOutput")
        dbg_part = nc.dram_tensor("dbg_part", [T, C], F32,
                                  kind="ExternalOutput")

    masks_dram = nc.inline_tensor(_mask_data(), name="cmasks")

    with tile.TileContext(nc) as tc:
        with (
            tc.tile_pool(name="const", bufs=1) as cp,
            tc.tile_pool(name="persist", bufs=1) as pp,
            tc.tile_pool(name="work", bufs=3) as wk_p,
            tc.tile_pool(name="dram", bufs=1, space="DRAM") as dp,
        ):
            # ---- constants (issued after the first xT/wq chunks below) ----
            masks = cp.tile([P, P], F32)
            bq_col = cp.tile([P, 2], F32)
            bk_col = cp.tile([P, 2], F32)
            bv_row = cp.tile([1, DL], F32)
            bo4_row = cp.tile([1, C], F32)

            def load_consts():
                nc.sync.dma_start(masks[:], masks_dram[:])
                nc.sync.dma_start(bq_col[:],
                                  bq[0, :].rearrange("(m p) -> p m", p=P))
                nc.sync.dma_start(bk_col[:],
                                  bk[0, :].rearrange("(m p) -> p m", p=P))
                nc.sync.dma_start(bv_row[:], bv[:])
                nc.sync.dma_start(bo4_row[:], bo4[:])

            bv_bc = cp.tile([P, DL], F32)
            bo4_bc = cp.tile([P, C], F32)

            # ---- persistent activations (bf16: halves SBUF traffic and
            # keeps all attention matmuls on the fast 2-byte PE path) ----
            qT_sb = pp.tile([P, 2, T], BF16)     # [d, t], d = mi*128+p
            kT_sb = pp.tile([P, 2, T], BF16)
            v_sb = pp.tile([P, NT, HL * (D + 1)], BF16)  # per head: 64 v + ones
            aoT_sb = pp.tile([P, 2, T], BF16)    # attention out^T (normalized)
            wo_sb = pp.tile([P, 2, C], BF16)

            # ones columns of v_sb (softmax denominator accumulator)
            ones64 = cp.tile([P, NT * HL], F32)
            nc.vector.memset(ones64[:], 1.0)
            vones = v_sb.rearrange("p n (h e) -> p n h e", h=HL)[:, :, :, D:D + 1]
            nc.vector.tensor_copy(vones, ones64[:].rearrange(
                "p (n h) -> p n h", n=NT)[:, :, :, None])

            # x / projection weights stay resident the whole kernel; the
            # projection itself is emitted as drain-filler jobs inside the
            # attention stream (shared "op" psum tag), so the exp pipeline
            # starts ~20us in instead of after a serial phase A.
            xTr = pp.tile([P, KC, T], BF16)
            wq_sb = pp.tile([P, KC, DL], BF16)
            wk_sb = pp.tile([P, KC, DL], BF16)
            wv_sb = pp.tile([P, KC, DL], BF16)
            # few large contiguous descriptors, ordered so the first qk
            # projection can start ~7us in; xT quartered so kk0-1 matmuls
            # chase the stream
            # wk after xT: k00 runs after q00 on the PE anyway, so only wq
            # needs to precede the x stream
            nc.sync.dma_start(wq_sb[:], wq[:])
            nc.sync.dma_start(
                xTr[:, 0:2, :],
                xT[0:2 * P, :].rearrange("(k p) t -> p k t", p=P))
            load_consts()
            nc.gpsimd.partition_broadcast(bv_bc[:], bv_row[:])
            nc.gpsimd.partition_broadcast(bo4_bc[:], bo4_row[:])
            for qtr in range(1, 4):
                nc.sync.dma_start(
                    xTr[:, 2 * qtr:2 * qtr + 2, :],
                    xT[2 * qtr * P:(2 * qtr + 2) * P, :].rearrange(
                        "(k p) t -> p k t", p=P))
            nc.sync.dma_start(wk_sb[:], wk[:])
            nc.sync.dma_start(wv_sb[:], wv[:])
            # wo needed only ~45us in; load after the proj weights
            nc.sync.dma_start(wo_sb[:], wo[:])

            v4 = v_sb.rearrange("p n (h e) -> p n h e", h=HL)
            bv4 = bv_bc.rearrange("p (h e) -> p h e", e=D)

            with (
                tc.tile_pool(name="ps_sT", bufs=4, space="PSUM") as ps_sT,
                tc.tile_pool(name="ps_pv", bufs=2, space="PSUM") as ps_pv,
                tc.tile_pool(name="ps_op", bufs=2, space="PSUM") as ps_op,
            ):
                # one part tile per chunk: the RS read of chunk qc-1 must
                # not alias the part writes of chunk qc (whole-tile WAR
                # tracking would stall the write DMAs behind the collective)
                parts = [dp.tile([QB, C], BF16, name=f"part{qc}")
                         for qc in range(NQ)]
                rs_out = dp.tile([NQ, P, C], BF16)

                def rs_job(qc):
                    def job():
                        # fire-and-forget: the DRAM->out copies ride the
                        # gpsimd queue (scheduler places copy qc after RS qc,
                        # whose wait resolves before anything queued behind
                        # it is data-ready), so nothing head-blocks on
                        # collective latency
                        nc.gpsimd.collective_compute(
                            "ReduceScatter",
                            mybir.AluOpType.add,
                            replica_groups=[[0, 1, 2, 3], [4, 5, 6, 7]],
                            ins=[parts[qc][:]],
                            outs=[rs_out[qc]],
                        )
                    return job

                def copy_job(qc):
                    def job():
                        nc.sync.dma_start(out[qc], rs_out[qc])
                    return job

                def qk_job(dst, w_sb, b_col, mi, ni):
                    # one [128, 512] slice of the q or k projection
                    def job():
                        ps = ps_op.tile([P, QB], F32, name="op_ps", tag="op")
                        for kk in range(KC):
                            nc.tensor.matmul(
                                ps[:],
                                w_sb[:, kk, mi * P:(mi + 1) * P],
                                xTr[:, kk, ni * QB:(ni + 1) * QB],
                                start=(kk == 0), stop=(kk == KC - 1))
                        nc.vector.tensor_scalar_add(
                            dst[:, mi, ni * QB:(ni + 1) * QB], ps[:],
                            b_col[:, mi:mi + 1])
                    return job

                def v_job(ti):
                    # one 128-token tile of the v projection
                    def job():
                        ps = ps_op.tile([P, QB], F32, name="op_ps", tag="op")
                        for kk in range(KC):
                            nc.tensor.matmul(
                                ps[:, 0:DL],
                                xTr[:, kk, ti * P:(ti + 1) * P],
                                wv_sb[:, kk, :],
                                start=(kk == 0), stop=(kk == KC - 1))
                        nc.vector.tensor_add(
                            v4[:, ti, :, 0:D],
                            ps[:, 0:DL].rearrange("p (h e) -> p h e", e=D),
                            bv4)
                    return job

                def outproj_jobs(qc):
                    # 8 projection psum-groups + deferred RS for chunk qc;
                    # emitted one at a time inside the NEXT chunk's attention
                    # stream as exp-independent PE gap filler.
                    jobs = []

                    def group(r, ni):
                        mi2 = 4 * qc + r
                        ps = ps_op.tile([P, QB], F32, name="op_ps", tag="op")
                        for kk2 in range(2):
                            nc.tensor.matmul(
                                ps[:],
                                aoT_sb[:, kk2, mi2 * P:(mi2 + 1) * P],
                                wo_sb[:, kk2, ni * QB:(ni + 1) * QB],
                                start=(kk2 == 0), stop=(kk2 == 1))
                        o_sb = wk_p.tile([P, QB], BF16, name="o_sb",
                                         tag="o_sb", bufs=10)
                        nc.vector.tensor_add(
                            o_sb[:], ps[:], bo4_bc[:, ni * QB:(ni + 1) * QB])
                        nc.gpsimd.dma_start(
                            parts[qc][r * P:(r + 1) * P,
                                      ni * QB:(ni + 1) * QB],
                            o_sb[:])

                    for r in range(4):
                        for ni in range(2):
                            jobs.append(lambda r=r, ni=ni: group(r, ni))
                    jobs.append(rs_job(qc))
                    return jobs

                # split variant for the last chunk: the mi=0 half of the
                # projection runs as drain filler during the second head
                # pair's attention; only the mi=1 half lands in the tail
                o0_sb = pp.tile([P, 8, QB], BF16)

                def outproj_a_jobs(qc):
                    jobs = []

                    def group_a(g):
                        r, ni = g // 2, g % 2
                        mi2 = 4 * qc + r
                        ps = ps_op.tile([P, QB], F32, name="op_ps", tag="op")
                        nc.tensor.matmul(
                            ps[:],
                            aoT_sb[:, 0, mi2 * P:(mi2 + 1) * P],
                            wo_sb[:, 0, ni * QB:(ni + 1) * QB],
                            start=True, stop=True)
                        nc.vector.tensor_add(
                            o0_sb[:, g, :], ps[:],
                            bo4_bc[:, ni * QB:(ni + 1) * QB])

                    for g in range(8):
                        jobs.append(lambda g=g: group_a(g))
                    return jobs

                def outproj_b(qc):
                    # tail: hw-DGE (sync) writes — the DMA rings are quiet
                    # here (all mid-kernel collectives done), so the parts
                    # land fast and the final RS starts with minimal lag
                    for g in (0, 1, 2, 3, 4, 5, 6, 7):
                        r, ni = g // 2, g % 2
                        mi2 = 4 * qc + r
                        ps = ps_op.tile([P, QB], F32, name="op_ps", tag="op")
                        nc.tensor.matmul(
                            ps[:],
                            aoT_sb[:, 1, mi2 * P:(mi2 + 1) * P],
                            wo_sb[:, 1, ni * QB:(ni + 1) * QB],
                            start=True, stop=True)
                        o_sb = wk_p.tile([P, QB], BF16, name="o_sb",
                                         tag="o_sb", bufs=10)
                        nc.vector.tensor_add(o_sb[:], ps[:], o0_sb[:, g, :])
                        nc.sync.dma_start(
                            parts[qc][r * P:(r + 1) * P,
                                      ni * QB:(ni + 1) * QB],
                            o_sb[:])
                    rs_job(qc)()

                pending = []

                def drain():
                    if pending:
                        pending.pop(0)()

                # pre-attention inline: just enough projection for chunk 0's
                # first head pair (v2/v3 aren't needed until diag 2-3's pv,
                # so they drain); everything else drains inside attention
                qk_job(qT_sb, wq_sb, bq_col, 0, 0)()
                qk_job(kT_sb, wk_sb, bk_col, 0, 0)()
                v_job(0)()
                v_job(1)()
                pending.extend([
                    v_job(2), v_job(3),
                    qk_job(qT_sb, wq_sb, bq_col, 1, 0),
                    qk_job(kT_sb, wk_sb, bk_col, 1, 0),
                    qk_job(qT_sb, wq_sb, bq_col, 0, 1),
                    qk_job(kT_sb, wk_sb, bk_col, 0, 1),
                    v_job(4), v_job(5),
                ])
                carry = []

                for qc in range(NQ):
                    qlo, qhi = qc * QB, (qc + 1) * QB
                    nkb = 4 * qc
                    for hp in range(2):
                        mi = hp
                        h0, h1 = 2 * hp, 2 * hp + 1
                        pv0 = ps_pv.tile([P, QB], F32, name="pv_ps", tag="pv")
                        pv1 = ps_pv.tile([P, QB], F32, name="pv_ps", tag="pv")
                        emit_pv_prev = None

                        def make_full_pv(kb, pT, pv0=pv0, pv1=pv1, h0=h0,
                                         h1=h1):
                            def emit():
                                nc.tensor.matmul(
                                    pv0[0:D + 1, :],
                                    v_sb[:, kb, h0 * (D + 1):(h0 + 1) * (D + 1)],
                                    pT[:, 0, :],
                                    start=(kb == 0), stop=False)
                                nc.tensor.matmul(
                                    pv1[0:D + 1, :],
                                    v_sb[:, kb, h1 * (D + 1):(h1 + 1) * (D + 1)],
                                    pT[:, 1, :],
                                    start=(kb == 0), stop=False)
                            return emit

                        # full (unmasked) key blocks: per-head single-bank
                        # score tiles + per-head exps, so each psum slot
                        # frees after its own head's exp and the exp stream
                        # starts before the second head's score matmul
                        for kb in range(nkb):
                            sT0 = ps_sT.tile([P, QB], F32, name="sT_ps",
                                             tag="sT")
                            sT1 = ps_sT.tile([P, QB], F32, name="sT_ps",
                                             tag="sT")
                            nc.tensor.matmul(
                                sT0[:],
                                kT_sb[0:D, mi, kb * P:(kb + 1) * P],
                                qT_sb[0:D, mi, qlo:qhi],
                                start=True, stop=True)
                            if emit_pv_prev is not None:
                                emit_pv_prev()
                            pT = wk_p.tile([P, 2, QB], BF16, name="pT",
                                           tag="pT", bufs=4)
                            nc.scalar.activation(
                                pT[:, 0, :], sT0[:],
                                mybir.ActivationFunctionType.Exp,
                                scale=SCALE)
                            nc.tensor.matmul(
                                sT1[:],
                                kT_sb[D:2 * D, mi, kb * P:(kb + 1) * P],
                                qT_sb[D:2 * D, mi, qlo:qhi],
                                start=True, stop=True)
                            nc.scalar.activation(
                                pT[:, 1, :], sT1[:],
                                mybir.ActivationFunctionType.Exp,
                                scale=SCALE)
                            emit_pv_prev = make_full_pv(kb, pT)
                            drain()

                        # diagonal blocks, queries narrowed to the visible
                        # range [128*di, 512); only a [128,128] tril masked
                        for di in range(4):
                            kb = nkb + di
                            q0 = di * P          # first visible query col
                            qw = QB - q0
                            sT0 = ps_sT.tile([P, QB], F32, name="sT_ps",
                                             tag="sT")
                            sT1 = ps_sT.tile([P, QB], F32, name="sT_ps",
                                             tag="sT")
                            nc.tensor.matmul(
                                sT0[:, 0:qw],
                                kT_sb[0:D, mi, kb * P:(kb + 1) * P],
                                qT_sb[0:D, mi, qlo + q0:qhi],
                                start=True, stop=True)
                            if emit_pv_prev is not None:
                                emit_pv_prev()
                                emit_pv_prev = None
                            nc.vector.tensor_add(
                                sT0[:, 0:P], sT0[:, 0:P], masks[:])
                            pT = wk_p.tile([P, 2, QB], BF16, name="pT",
                                           tag="pT", bufs=4)
                            nc.scalar.activation(
                                pT[:, 0, 0:qw], sT0[:, 0:qw],
                                mybir.ActivationFunctionType.Exp,
                                scale=SCALE)
                            nc.tensor.matmul(
                                sT1[:, 0:qw],
                                kT_sb[D:2 * D, mi, kb * P:(kb + 1) * P],
                                qT_sb[D:2 * D, mi, qlo + q0:qhi],
                                start=True, stop=True)
                            nc.vector.tensor_add(
                                sT1[:, 0:P], sT1[:, 0:P], masks[:])
                            nc.scalar.activation(
                                pT[:, 1, 0:qw], sT1[:, 0:qw],
                                mybir.ActivationFunctionType.Exp,
                                scale=SCALE)

                            def emit_diag_pv(kb=kb, pT=pT, q0=q0, qw=qw,
                                             di=di, pv0=pv0, pv1=pv1,
                                             h0=h0, h1=h1):
                                nc.tensor.matmul(
                                    pv0[0:D + 1, q0:QB],
                                    v_sb[:, kb, h0 * (D + 1):(h0 + 1) * (D + 1)],
                                    pT[:, 0, 0:qw],
                                    start=(nkb == 0 and di == 0),
                                    stop=(di == 3))
                                nc.tensor.matmul(
                                    pv1[0:D + 1, q0:QB],
                                    v_sb[:, kb, h1 * (D + 1):(h1 + 1) * (D + 1)],
                                    pT[:, 1, 0:qw],
                                    start=(nkb == 0 and di == 0),
                                    stop=(di == 3))
                            emit_pv_prev = emit_diag_pv
                            drain()
                        emit_pv_prev()

                        # normalize: aoT = pv[0:D] / pv[D] (ones-row sum)
                        for h, pv in ((h0, pv0), (h1, pv1)):
                            po = 64 * (h % 2)
                            rbc = wk_p.tile([D, QB], F32, name="rbc",
                                            tag="rbc", bufs=2)
                            lrow = wk_p.tile([1, QB], F32, name="lrow",
                                             tag="lrow", bufs=2)
                            nc.vector.tensor_copy(lrow[:], pv[D:D + 1, :])
                            nc.vector.reciprocal_approx_fast(
                                out=rbc[0:1, :], in_=lrow[:])
                            nc.gpsimd.partition_broadcast(rbc[:], rbc[0:1, :])
                            nc.vector.tensor_mul(
                                aoT_sb[po:po + D, mi, qlo:qhi],
                                pv[0:D, :], rbc[:])

                        if qc == NQ - 1 and hp == 0:
                            # last chunk: chunk 2's held-back groups + RS,
                            # then the kk2=0 half of this chunk's projection
                            # drain during hp1's attention
                            pending.extend(carry)
                            carry.clear()
                            pending.extend(outproj_a_jobs(qc))

                    # queue work for the next chunk's attention stream:
                    # remaining projection slices (ordered so hp1's q/k and
                    # the next chunk's q/k emit before their consumers),
                    # then this chunk's output projection + RS
                    if qc == 0:
                        pending.extend([
                            v_job(6), v_job(7),
                            qk_job(qT_sb, wq_sb, bq_col, 1, 1),
                            qk_job(kT_sb, wk_sb, bk_col, 1, 1),
                            qk_job(qT_sb, wq_sb, bq_col, 0, 2),
                            qk_job(kT_sb, wk_sb, bk_col, 0, 2),
                            v_job(8), v_job(9), v_job(10),
                        ])
                        pending.extend(outproj_jobs(qc))
                    elif qc == 1:
                        pending.extend([
                            v_job(11),
                            qk_job(qT_sb, wq_sb, bq_col, 1, 2),
                            qk_job(kT_sb, wk_sb, bk_col, 1, 2),
                            qk_job(qT_sb, wq_sb, bq_col, 0, 3),
                            qk_job(kT_sb, wk_sb, bk_col, 0, 3),
                            v_job(12), v_job(13), v_job(14), v_job(15),
                        ])
                        pending.extend(outproj_jobs(qc))
                    elif qc == 2:
                        # hold back part of chunk 2's output projection for
                        # qc3-hp1, whose own drain list would otherwise run
                        # dry 8 slots early
                        j2 = outproj_jobs(qc)
                        pending.extend([
                            qk_job(qT_sb, wq_sb, bq_col, 1, 3),
                            qk_job(kT_sb, wk_sb, bk_col, 1, 3),
                        ])
                        pending.extend(j2[:6])
                        carry[:] = j2[6:]

                # drain leftovers, finish the last chunk's kk2=1 half with
                # split half-size RS ops, then write all chunks out
                while pending:
                    drain()
                outproj_b(NQ - 1)
                for qc in range(NQ):
                    copy_job(qc)()

                if debug_taps:
                    nc.sync.dma_start(dbg_q[:], qT_sb[:].bitcast(F32))
                    nc.sync.dma_start(dbg_k[:], kT_sb[:].bitcast(F32))
                    nc.sync.dma_start(dbg_v[:], v_sb[:].bitcast(F32))
                    ao_f = wk_p.tile([P, 2, T], F32, name="ao_f", tag="ao_f",
                                     bufs=1)
                    nc.vector.tensor_copy(ao_f[:], aoT_sb[:])
                    nc.sync.dma_start(dbg_ao[:], ao_f[:])
                    # part_dram is bf16 [T, C]; reload and upcast in chunks
                    for r in range(NT):
                        pb = wk_p.tile([P, C], BF16, name="pb", tag="pb",
                                       bufs=2)
                        nc.sync.dma_start(pb[:], part_dram[r * P:(r + 1) * P, :])
                        pf = wk_p.tile([P, C], F32, name="pf", tag="pf",
                                       bufs=2)
                        nc.vector.tensor_copy(pf[:], pb[:])
                        nc.sync.dma_start(dbg_part[r * P:(r + 1) * P, :], pf[:])

    nc.compile()
    return nc


def kernel(x, w_qkv, b_qkv, w_out, b_out):
    x = np.ascontiguousarray(np.asarray(x, dtype=np.float32))
    w_qkv = np.asarray(w_qkv, dtype=np.float32)
    b_qkv = np.asarray(b_qkv, dtype=np.float32)
    w_out = np.ascontiguousarray(np.asarray(w_out, dtype=np.float32))
    b_out = np.asarray(b_out, dtype=np.float32)

    if "nc" not in _CACHED:
        _CACHED["nc"] = _build()
    nc = _CACHED["nc"]

    BF = ml_dtypes.bfloat16

    def wlayout(w):
        # [C, DL] -> [P, KC*DL]: row kk*P+p lands at partition p, block kk
        return np.ascontiguousarray(
            w.reshape(KC, P, DL).transpose(1, 0, 2).reshape(P, KC * DL)
        ).astype(BF)

    xTs = [np.ascontiguousarray(x[b_].T).astype(BF) for b_ in range(B)]
    bo4 = np.ascontiguousarray((b_out / G)[None, :]).astype(np.float32)
    in_maps = []
    for c in range(8):
        b_, g = c // 4, c % 4
        sl = slice(g * DL, (g + 1) * DL)
        wo_l = w_out[g * DL:(g + 1) * DL, :]       # [DL, C]
        in_maps.append({
            "xT": xTs[b_],
            "wq": wlayout(w_qkv[:, 0 * C:1 * C][:, sl]),
            "wk": wlayout(w_qkv[:, 1 * C:2 * C][:, sl]),
            "wv": wlayout(w_qkv[:, 2 * C:3 * C][:, sl]),
            "bq": np.ascontiguousarray(b_qkv[0 * C:1 * C][sl][None, :]),
            "bk": np.ascontiguousarray(b_qkv[1 * C:2 * C][sl][None, :]),
            "bv": np.ascontiguousarray(b_qkv[2 * C:3 * C][sl][None, :]),
            "wo": np.ascontiguousarray(
                wo_l.reshape(2, P, C).transpose(1, 0, 2).reshape(P, 2 * C)
            ).astype(BF),
            "bo4": bo4,
        })
    res = run_bass_kernel_spmd(nc, in_maps, list(range(8)))
    global LAST_RESULT
    LAST_RESULT = res
    out_full = np.empty((B, T, C), dtype=np.float32)
    for c in range(8):
        b_, g = c // 4, c % 4
        o = res.results[c]["out"]          # [NQ, P, C] bf16
        for qc in range(NQ - 1):
            r0 = qc * QB + g * P
            out_full[b_, r0:r0 + P, :] = o[qc].astype(np.float32)
        qc = NQ - 1
        r0 = qc * QB + g * P
        out_full[b_, r0:r0 + P, :] = o[qc].astype(np.float32)
    return out_full

